# revision 6
# baseline (speedup 1.0000x reference)
"""Trainium2 Bass kernel for nn_ClassCenters (pairwise squared L2 distances).

dist[n, c] = relu(||e_n||^2 + ||c_c||^2 - 2 e_n . c_c)   for
embedding [16384, 1024] f32, centers [1000, 1024] f32 -> [16384, 1000] f32.

Sharding: data-parallel over embedding rows, 8 cores x 2048 rows; centers
replicated.  Both operands are shipped transposed (layout-only) so the
contraction dim D sits on SBUF partitions as the matmul requires.

Per-core device program:
  - centers^T loaded first (per k-tile/n-chunk DMAs), embeddings^T streamed
    in m-blocks so PSUM groups complete while later blocks still load.
  - main matmul in float32r (full PE rate at N>=256, ~tf32 mantissa).
  - ynorm: DVE squares + ones-matmul partition reduction -> row [1, C],
    then a K=1 fp32 matmul broadcasts -0.5*ynorm to all 128 partitions.
  - xnorm: per-m-tile ones-matmul (N=1) into PSUM [128,1], copied to an
    SBUF column -> used as the per-partition activation bias.
  - epilogue per (m-tile, n-chunk): DVE t = psum + (-0.5*ynorm)bcast, then
    ACT out = Relu(-2*t + xnorm) and one row-contiguous output DMA per m-tile.
"""
import sys

sys.path.insert(0, "/opt/trn_rl_repo")
import numpy as np

N_TOTAL, C, D = 16384, 1000, 1024
NCORES = 8
NS = N_TOTAL // NCORES  # 2048 rows per core
KT = D // 128  # 8 contraction tiles
MB = 2  # m-tiles (128 rows) per emb block
NCH = ((0, 512), (512, 488))  # n-chunks of C

_CACHE = {}


def build_nc(ns=NS):
    import concourse.mybir as mybir
    import concourse.tile as tile
    import concourse.bacc as bacc

    F32, F32R = mybir.dt.float32, mybir.dt.float32r
    AL = mybir.AluOpType
    AF = mybir.ActivationFunctionType

    mt_total = ns // 128
    nblk = mt_total // MB

    nc = bacc.Bacc(None, target_bir_lowering=False)
    embT = nc.declare_dram_parameter("embT", [D, ns], F32R, isOutput=False)
    cenT = nc.declare_dram_parameter("cenT", [D, C], F32R, isOutput=False)
    out = nc.declare_dram_parameter("out", [ns, C], F32, isOutput=True)

    ebd = embT.rearrange("(kt p) m -> kt p m", p=128)
    ced = cenT.rearrange("(kt p) c -> kt p c", p=128)

    with tile.TileContext(nc) as tc:
        with (
            tc.tile_pool(name="const", bufs=1) as constp,
            tc.tile_pool(name="cen", bufs=1) as cenp,
            tc.tile_pool(name="rows", bufs=1) as rowp,
            tc.tile_pool(name="emb", bufs=3) as embp,
            tc.tile_pool(name="sq", bufs=2) as sqp,
            tc.tile_pool(name="eplg", bufs=4) as ep,
            tc.tile_pool(name="outp", bufs=3) as otp,
        ):
            # f32r matmuls require even innermost free counts on the moving
            # operand and dst, so the ones helper is 2 columns wide.
            ones2 = constp.tile([128, 2], F32)
            nc.gpsimd.memset(ones2[:], 1.0)
            ones2_r = ones2[:].bitcast(F32R)
            nhalf = constp.tile([1, 128], F32)
            nc.gpsimd.memset(nhalf[:], -0.5)

            # ---- centers: load n-chunk 0 for all k first, then n-chunk 1
            ce = cenp.tile([128, KT, C], F32R)
            for o, w in NCH:
                for k in range(KT):
                    nc.sync.dma_start(ce[:, k, o : o + w], ced[k, :, o : o + w])

            ynr = rowp.tile([1, C], F32)
            ybc = rowp.tile([128, C], F32)
            xnc = rowp.tile([128, mt_total], F32)

            # ---- ynorm: squares + partition-reduce + broadcast(-0.5*)
            with tc.tile_pool(name="psy", bufs=1, space="PSUM") as psy:
                ps_y = {o: psy.tile([2, w], F32, name=f"ps_y{o}") for o, w in NCH}
                for k in range(KT):
                    sqc = sqp.tile([128, C], F32R, name=f"sqc{k}", tag="sqc")
                    nc.vector.tensor_tensor(
                        sqc[:], ce[:, k, :].bitcast(F32), ce[:, k, :].bitcast(F32),
                        op=AL.mult,
                    )
                    for o, w in NCH:
                        nc.tensor.matmul(
                            ps_y[o][:], ones2_r, sqc[:, o : o + w],
                            start=(k == 0), stop=(k == KT - 1),
                        )
                for o, w in NCH:
                    nc.vector.tensor_copy(ynr[:, o : o + w], ps_y[o][0:1, :])
                ps_b = {o: psy.tile([128, w], F32, name=f"ps_b{o}") for o, w in NCH}
                for o, w in NCH:
                    nc.tensor.matmul(ps_b[o][:], nhalf[:], ynr[:1, o : o + w])
                    nc.vector.tensor_copy(ybc[:, o : o + w], ps_b[o][:])

            # ---- main: emb blocks stream in; per-block xnorm + matmul + epilogue
            with (
                tc.tile_pool(name="psm", bufs=2, space="PSUM") as psm,
                tc.tile_pool(name="psx", bufs=2, space="PSUM") as psx,
            ):
                for b in range(nblk):
                    mlo = b * MB * 128
                    eb = embp.tile([128, KT, MB * 128], F32R, name=f"eb{b}", tag="eb")
                    for k in range(KT):
                        nc.sync.dma_start(eb[:, k, :], ebd[k, :, mlo : mlo + MB * 128])

                    # xnorm for the block's m-tiles
                    px = [
                        psx.tile([128, 2], F32, name=f"px{b}_{j}", tag="px")
                        for j in range(MB)
                    ]
                    for k in range(KT):
                        sqe = sqp.tile(
                            [128, MB * 128], F32R, name=f"sqe{b}_{k}", tag="sqe"
                        )
                        nc.vector.tensor_tensor(
                            sqe[:], eb[:, k, :].bitcast(F32), eb[:, k, :].bitcast(F32),
                            op=AL.mult,
                        )
                        for j in range(MB):
                            nc.tensor.matmul(
                                px[j][:], sqe[:, j * 128 : (j + 1) * 128], ones2_r,
                                start=(k == 0), stop=(k == KT - 1),
                            )
                    for j in range(MB):
                        mt = b * MB + j
                        nc.scalar.activation(
                            xnc[:, mt : mt + 1], px[j][:, 0:1], AF.Copy
                        )

                    # main matmul + epilogue per m-tile
                    for j in range(MB):
                        mt = b * MB + j
                        ot = otp.tile([128, C], F32, name=f"ot{mt}", tag="ot")
                        for o, w in NCH:
                            ps = psm.tile(
                                [128, w], F32, name=f"ps{mt}_{o}", tag=f"ps{o}"
                            )
                            for k in range(KT):
                                nc.tensor.matmul(
                                    ps[:],
                                    eb[:, k, j * 128 : (j + 1) * 128],
                                    ce[:, k, o : o + w],
                                    start=(k == 0), stop=(k == KT - 1),
                                )
                            t = ep.tile([128, w], F32, name=f"t{mt}_{o}", tag=f"t{o}")
                            nc.vector.scalar_tensor_tensor(
                                t[:], ps[:], 0.0, ybc[:, o : o + w],
                                op0=AL.add, op1=AL.add,
                            )
                            nc.scalar.activation(
                                ot[:, o : o + w], t[:], AF.Relu,
                                bias=xnc[:, mt : mt + 1], scale=-2.0,
                            )
                        nc.scalar.dma_start(out[mt * 128 : (mt + 1) * 128, :], ot[:])
    nc.compile()
    return nc


def kernel(embedding: np.ndarray, centers: np.ndarray) -> np.ndarray:
    from concourse.bass_utils import run_bass_kernel_spmd

    if "nc" not in _CACHE:
        _CACHE["nc"] = build_nc()
    nc = _CACHE["nc"]

    embT = np.ascontiguousarray(embedding.T)  # [D, N]
    cenT = np.ascontiguousarray(centers.astype(np.float32).T)  # [D, C]
    in_maps = [
        {
            "embT": np.ascontiguousarray(embT[:, c * NS : (c + 1) * NS]),
            "cenT": cenT,
        }
        for c in range(NCORES)
    ]
    res = run_bass_kernel_spmd(nc, in_maps, core_ids=list(range(NCORES)))
    return np.concatenate([r["out"] for r in res.results], axis=0)
